# revision 3
# baseline (speedup 1.0000x reference)
"""Multi-head attention (S=2048, E=2048, H=16, D=128) on 8 NeuronCores.

Sharding: tensor-parallel over heads. Core i owns heads {2i, 2i+1}:
 - Wq/Wk/Wv split column-wise (256 output features per core)
 - each core computes its heads' scores/softmax/AV locally
 - Wo split row-wise; each core emits a partial [S, E] output (transposed);
   host sums the 8 partials and adds bo.

Layouts (per core), everything "T" = transposed so the contraction dim
lands on SBUF partitions:
 - xT   [E, S]      x transposed (host)
 - wq/wk [128, 16, 256]  Wq_local.T chunked: [e%128, e//128, f_local]
 - wv   [128, 16, 256]   same layout (used as matmul rhs)
 - wo   [128, 2, 2048]   Wo_local.T chunked: [f%128, head, j]
 - QT/KT [128, 2, S]     [d, head, s] — head-dim on partitions
 - V    [128, 16, 256]   [t%128, t//128, f_local] — seq on partitions
 - attnT [128, 16, 512]  exp(scores.T) for one head and one 512-col s-block
 - outT [E, S]           partial output, transposed

Matmuls run in float32r (full-rate fp32 mode on TRN2 PE, ~1.5e-4 rel err);
set MATMUL_FP32R = False to fall back to exact-fp32 (4x slower) matmuls.
"""

import sys

sys.path.insert(0, "/opt/trn_rl_repo")

import numpy as np

import concourse.bass as bass
import concourse.mybir as mybir
import concourse.tile as tile
from concourse import bacc
from concourse.bass_utils import run_bass_kernel_spmd

F32 = mybir.dt.float32
F32R = mybir.dt.float32r
ActFn = mybir.ActivationFunctionType

S = 2048
E = 2048
H = 16
D = 128
NCORES = 8
FH = E // NCORES          # local output features = 256 (2 heads)
HPC = FH // D             # heads per core = 2
SCALE = float(1.0 / np.sqrt(D))

MATMUL_FP32R = True
MMDT = F32R if MATMUL_FP32R else F32

_nc_cache = None
last_results = None       # set by kernel(); test harness reads exec_time_ns


def _build():
    nc = bacc.Bacc(None, target_bir_lowering=False, debug=False)

    xT = nc.dram_tensor("xT", [E, S], MMDT, kind="ExternalInput")
    wq = nc.dram_tensor("wq", [128, 16, FH], MMDT, kind="ExternalInput")
    wk = nc.dram_tensor("wk", [128, 16, FH], MMDT, kind="ExternalInput")
    wv = nc.dram_tensor("wv", [128, 16, FH], MMDT, kind="ExternalInput")
    wo = nc.dram_tensor("wo", [128, HPC, E], MMDT, kind="ExternalInput")
    bq = nc.dram_tensor("bq", [128, HPC], F32, kind="ExternalInput")
    bk = nc.dram_tensor("bk", [128, HPC], F32, kind="ExternalInput")
    bv = nc.dram_tensor("bv", [1, FH], F32, kind="ExternalInput")
    outT = nc.dram_tensor("outT", [E, S], F32, kind="ExternalOutput")

    with tile.TileContext(nc) as tc:
        with (
            tc.tile_pool(name="weights", bufs=1) as wpool,
            tc.tile_pool(name="qkv", bufs=1) as qkvpool,
            tc.tile_pool(name="consts", bufs=1) as cpool,
        ):
            wq_sb = wpool.tile([128, 16, FH], MMDT)
            wk_sb = wpool.tile([128, 16, FH], MMDT)
            wv_sb = wpool.tile([128, 16, FH], MMDT)
            wo_sb = wpool.tile([128, HPC, E], MMDT)
            nc.sync.dma_start(wq_sb[:], wq[:])
            nc.sync.dma_start(wk_sb[:], wk[:])
            nc.sync.dma_start(wv_sb[:], wv[:])
            nc.sync.dma_start(wo_sb[:], wo[:])

            bq_sb = cpool.tile([128, HPC], F32)
            bk_sb = cpool.tile([128, HPC], F32)
            bv_bc = cpool.tile([128, FH], F32)
            ones = cpool.tile([128, 128], F32)
            nc.sync.dma_start(bq_sb[:], bq[:])
            nc.sync.dma_start(bk_sb[:], bk[:])
            nc.sync.dma_start(bv_bc[:], bv[:].to_broadcast((128, FH)))
            nc.vector.memset(ones[:], 1.0)

            qt_sb = qkvpool.tile([128, HPC, S], MMDT)   # [d, h, s]
            kt_sb = qkvpool.tile([128, HPC, S], MMDT)   # [d, h, t]
            v_sb = qkvpool.tile([128, 16, FH], MMDT)    # [t%128, t//128, f]

            # ---------------- Phase A: Q/K/V projections ----------------
            with (
                tc.tile_pool(name="xt", bufs=1) as xpool,
                tc.tile_pool(name="psa", bufs=8, space="PSUM") as psa,
            ):
                for half in range(2):
                    xt = xpool.tile([128, 16, 1024], MMDT, tag="xt")
                    for e in range(16):
                        nc.sync.dma_start(
                            xt[:, e, :],
                            xT[e * 128:(e + 1) * 128,
                               half * 1024:(half + 1) * 1024],
                        )
                    # Q/K: out[f, s] accumulated over e; 8 PSUM banks live
                    qk_ps = {}
                    for ft in range(HPC):
                        for st in range(2):
                            qk_ps[("q", ft, st)] = psa.tile(
                                [128, 512], F32, tag="psa",
                                name=f"q_ps_{half}_{ft}_{st}")
                            qk_ps[("k", ft, st)] = psa.tile(
                                [128, 512], F32, tag="psa",
                                name=f"k_ps_{half}_{ft}_{st}")
                    for e in range(16):
                        for ft in range(HPC):
                            lq = wq_sb[:, e, ft * 128:(ft + 1) * 128]
                            lk = wk_sb[:, e, ft * 128:(ft + 1) * 128]
                            for st in range(2):
                                rx = xt[:, e, st * 512:(st + 1) * 512]
                                nc.tensor.matmul(
                                    qk_ps[("q", ft, st)][:], lq, rx,
                                    start=(e == 0), stop=(e == 15))
                                nc.tensor.matmul(
                                    qk_ps[("k", ft, st)][:], lk, rx,
                                    start=(e == 0), stop=(e == 15))
                    for ft in range(HPC):
                        for st in range(2):
                            s_off = half * 1024 + st * 512
                            nc.scalar.activation(
                                qt_sb[:, ft, s_off:s_off + 512],
                                qk_ps[("q", ft, st)][:],
                                ActFn.Identity, bias=bq_sb[:, ft:ft + 1])
                            nc.scalar.activation(
                                kt_sb[:, ft, s_off:s_off + 512],
                                qk_ps[("k", ft, st)][:],
                                ActFn.Identity, bias=bk_sb[:, ft:ft + 1])
                    # V: out[t, f] accumulated over e (stationary = xT block)
                    for tt in range(8):
                        vp = psa.tile([128, FH], F32, tag="psa")
                        for e in range(16):
                            nc.tensor.matmul(
                                vp[:],
                                xt[:, e, tt * 128:(tt + 1) * 128],
                                wv_sb[:, e, :],
                                start=(e == 0), stop=(e == 15))
                        nc.vector.tensor_add(
                            v_sb[:, half * 8 + tt, :], vp[:], bv_bc[:])

            # ---------------- Phase B: attention + output proj ----------------
            with (
                tc.tile_pool(name="attn", bufs=2) as apool,
                tc.tile_pool(name="small", bufs=4) as spool,
                tc.tile_pool(name="ostage", bufs=3) as opool,
                tc.tile_pool(name="ps_s", bufs=3, space="PSUM") as ps_s,
                tc.tile_pool(name="ps_av", bufs=2, space="PSUM") as ps_av,
                tc.tile_pool(name="ps_sum", bufs=1, space="PSUM") as ps_sum,
                tc.tile_pool(name="ps_o", bufs=2, space="PSUM") as ps_o,
            ):
                for blk in range(4):
                    s_sl = slice(blk * 512, (blk + 1) * 512)
                    ao = []
                    for h in range(HPC):
                        aT = apool.tile([128, 16, 512], MMDT, tag="attnT")
                        acc = spool.tile([128, 512], F32, tag="acc")
                        for tcn in range(16):
                            sp = ps_s.tile([128, 512], F32, tag="s_ps")
                            nc.tensor.matmul(
                                sp[:],
                                kt_sb[:, h, tcn * 128:(tcn + 1) * 128],
                                qt_sb[:, h, s_sl],
                                start=True, stop=True)
                            nc.scalar.activation(
                                aT[:, tcn, :], sp[:], ActFn.Exp, scale=SCALE)
                            if tcn == 0:
                                nc.vector.tensor_copy(
                                    acc[:], aT[:, 0, :].bitcast(F32))
                            else:
                                nc.vector.tensor_add(
                                    acc[:], acc[:], aT[:, tcn, :].bitcast(F32))
                        av = ps_av.tile([128, 512], F32, tag="av_ps")
                        for tcn in range(16):
                            nc.tensor.matmul(
                                av[:],
                                v_sb[:, tcn, h * 128:(h + 1) * 128],
                                aT[:, tcn, :],
                                start=(tcn == 0), stop=(tcn == 15))
                        sm = ps_sum.tile([128, 512], F32, tag="sum_ps")
                        nc.tensor.matmul(sm[:], ones[:], acc[:],
                                         start=True, stop=True)
                        rcp = spool.tile([128, 512], F32, tag="rcp")
                        nc.vector.reciprocal(rcp[:], sm[:])
                        aoh = spool.tile([128, 512], MMDT, tag="ao")
                        nc.vector.tensor_mul(aoh[:], av[:], rcp[:])
                        ao.append(aoh)
                    for jt in range(16):
                        op = ps_o.tile([128, 512], F32, tag="o_ps")
                        for h in range(HPC):
                            nc.tensor.matmul(
                                op[:],
                                wo_sb[:, h, jt * 128:(jt + 1) * 128],
                                ao[h][:],
                                start=(h == 0), stop=(h == HPC - 1))
                        ost = opool.tile([128, 512], F32, tag="ost")
                        nc.vector.tensor_copy(ost[:], op[:])
                        nc.sync.dma_start(
                            outT[jt * 128:(jt + 1) * 128, s_sl], ost[:])
    nc.compile()
    return nc


def _get_nc():
    global _nc_cache
    if _nc_cache is None:
        _nc_cache = _build()
    return _nc_cache


def kernel(x, Wq, bq, Wk, bk, Wv, bv, Wo, bo):
    global last_results
    x = np.ascontiguousarray(np.asarray(x, dtype=np.float32))
    Wq = np.asarray(Wq, dtype=np.float32)
    Wk = np.asarray(Wk, dtype=np.float32)
    Wv = np.asarray(Wv, dtype=np.float32)
    Wo = np.asarray(Wo, dtype=np.float32)
    bq = np.asarray(bq, dtype=np.float32)
    bk = np.asarray(bk, dtype=np.float32)
    bv = np.asarray(bv, dtype=np.float32)
    bo = np.asarray(bo, dtype=np.float32)

    nc = _get_nc()
    xT = np.ascontiguousarray(x.T)

    def wslice(W, c):
        # Wq_local.T = W[c*FH:(c+1)*FH, :].T -> [E, FH] -> [128, 16, FH]
        wt = W[c * FH:(c + 1) * FH, :].T          # [E, FH]
        return np.ascontiguousarray(
            wt.reshape(16, 128, FH).transpose(1, 0, 2))

    in_maps = []
    for c in range(NCORES):
        fsl = slice(c * FH, (c + 1) * FH)
        wo_c = np.ascontiguousarray(
            Wo[:, fsl].T.reshape(HPC, 128, E).transpose(1, 0, 2))  # [128,2,E]
        in_maps.append({
            "xT": xT,
            "wq": wslice(Wq, c),
            "wk": wslice(Wk, c),
            "wv": wslice(Wv, c),
            "wo": wo_c,
            "bq": np.ascontiguousarray(bq[fsl].reshape(HPC, 128).T),
            "bk": np.ascontiguousarray(bk[fsl].reshape(HPC, 128).T),
            "bv": np.ascontiguousarray(bv[fsl].reshape(1, FH)),
        })

    res = run_bass_kernel_spmd(nc, in_maps, list(range(NCORES)))
    last_results = res

    acc = np.zeros((E, S), dtype=np.float64)
    for c in range(NCORES):
        acc += res.results[c]["outT"].astype(np.float64)
    out = acc.T + bo[None, :].astype(np.float64)
    return out.astype(np.float32)
